# revision 1
# baseline (speedup 1.0000x reference)
"""Paged KV-cache decode attention with ALiBi (Baichuan-style), fused
QKV + attention + output projection, tensor-parallel over heads across
8 Trainium2 NeuronCores.

Layout strategy (per core, 5 heads):
  - qT/kT computed as [640, 4] (head-dim on partitions) so scores matmuls
    need no transposes and the K-cache new-token scatter is a same-partition
    SBUF copy.
  - v computed as [4, 640] (natural) so the V new-token scatter is a tiny
    SBUF->SBUF DMA row write.
  - K cache staged host-side per core as [5, 4, 128(d), 2048(t)] (K^T),
    V cache as [5, 4, 128(t%128), 16(chunk), 128(d)] so every device DMA is
    a large (>=0.5-1MB) mostly-contiguous transfer.
  - softmax without max-subtraction (scores are O(10); exp is safe in fp32),
    masking baked into a host-precomputed additive bias (-1e30).
  - o_proj computed transposed (out^T [5120, 4]) per core; host sums the 8
    partial products (the "all-reduce").
"""

import math
import os
import sys
from contextlib import ExitStack

import numpy as np

sys.path.insert(0, "/opt/trn_rl_repo")

B = 4
E = 5120
H = 40
D = 128
BS = 16
NB = 512
MB = 128
S = MB * BS  # 2048
NCORES = 8
HPC = H // NCORES   # 5 heads per core
EPC = HPC * D       # 640

NEG = -1.0e30


def _alibi_slopes(num_heads):
    cp2 = 2 ** int(math.floor(math.log2(num_heads)))
    base = 2.0 ** (-(2.0 ** (-(math.log2(cp2) - 3))))
    slopes = base ** np.arange(1, cp2 + 1, dtype=np.float64)
    if cp2 != num_heads:
        extra_base = 2.0 ** (-(2.0 ** (-(math.log2(2 * cp2) - 3))))
        n_rem = min(cp2, num_heads - cp2)
        extra = extra_base ** np.arange(1, 1 + 2 * n_rem, 2, dtype=np.float64)
        slopes = np.concatenate([slopes, extra])
    return slopes.astype(np.float32)


_PROGRAM_CACHE = {}
LAST_RESULTS = None  # BassKernelResults of the most recent run (for test.py)


def _build_program(pos, nch):
    """Build the SPMD Bass program. pos/nch are per-sequence tuples, baked
    statically (same for all cores; per-core data varies only via inputs)."""
    import concourse.bacc as bacc
    import concourse.bass as bass
    import concourse.tile as tile
    from concourse import mybir

    f32 = mybir.dt.float32
    nc = bacc.Bacc()

    hT = nc.declare_dram_parameter("hT", [128, 40 * B], f32, isOutput=False)
    qkvw = nc.declare_dram_parameter("qkvw", [3, E, EPC], f32, isOutput=False)
    ow = nc.declare_dram_parameter("ow", [EPC, E], f32, isOutput=False)
    kt = nc.declare_dram_parameter("kt", [HPC, B, D, S], f32, isOutput=False)
    vt = nc.declare_dram_parameter("vt", [HPC, B, 128, 16, D], f32, isOutput=False)
    bias = nc.declare_dram_parameter("bias", [128, B * HPC * 16], f32, isOutput=False)
    outT = nc.declare_dram_parameter("outT", [128, 40 * B], f32, isOutput=True)

    with tile.TileContext(nc) as tc, ExitStack() as ctx:
        consts = ctx.enter_context(tc.tile_pool(name="consts", bufs=1))
        wpool = ctx.enter_context(tc.tile_pool(name="wpool", bufs=2))
        kvpool = ctx.enter_context(tc.tile_pool(name="kvpool", bufs=3))
        tmp = ctx.enter_context(tc.tile_pool(name="tmp", bufs=3))
        opool = ctx.enter_context(tc.tile_pool(name="opool", bufs=2))
        psum = ctx.enter_context(tc.tile_pool(name="psum", bufs=8, space="PSUM"))

        # ---- constants / small inputs ----
        hT_sb = consts.tile([128, 40 * B], f32)          # (E%128, (Echunk, b))
        nc.gpsimd.dma_start(out=hT_sb[:], in_=hT[:])
        bias_sb = consts.tile([128, B * HPC * 16], f32)  # (t%128, (b, h, chunk))
        nc.gpsimd.dma_start(out=bias_sb[:], in_=bias[:])
        ones_col = consts.tile([128, 1], f32)
        nc.vector.memset(ones_col[:], 1.0)
        ones_row = consts.tile([1, 128], f32)
        nc.vector.memset(ones_row[:], 1.0)

        qT_sb = consts.tile([128, HPC * B], f32)   # col = h*B + b ; partition = d
        kT_sb = consts.tile([128, HPC * B], f32)
        v_sb = consts.tile([B, EPC], f32)          # natural v rows
        colsum_sb = consts.tile([128, HPC * B], f32)
        aoT_sb = consts.tile([128, HPC * B], f32)  # unnormalized attn@V ^T
        outT_sb = consts.tile([128, 40 * B], f32)

        # ---- fused QKV projection ----
        # q,k transposed orientation: psum[oc] [128, B] accumulated over 40
        # E-chunks; lhsT = W chunk [128(E), 128(outcol)], rhs = hT chunk [128(E), B].
        for w in range(2):  # 0=q (pre-scaled on host), 1=k
            dst = qT_sb if w == 0 else kT_sb
            ps = [psum.tile([128, B], f32, tag="ps", name=f"ps_qk{w}_{i}") for i in range(HPC)]
            for g in range(10):  # groups of 4 E-chunks
                wt = wpool.tile([128, 4 * EPC], f32, tag="w")
                nc.gpsimd.dma_start(
                    out=wt[:],
                    in_=qkvw[w, g * 512:(g + 1) * 512, :].rearrange(
                        "(kl p) c -> p kl c", p=128
                    ),
                )
                for oc in range(HPC):
                    for kl in range(4):
                        kc = g * 4 + kl
                        nc.tensor.matmul(
                            ps[oc][:],
                            lhsT=wt[:, kl * EPC + oc * 128: kl * EPC + (oc + 1) * 128],
                            rhs=hT_sb[:, kc * B:(kc + 1) * B],
                            start=(kc == 0),
                            stop=(kc == 39),
                        )
            for oc in range(HPC):
                nc.scalar.copy(dst[:, oc * B:(oc + 1) * B], ps[oc][:])

        # v natural orientation: psum [B, 640] (two banks: 512 + 128),
        # lhsT = hT chunk [128(E), B], rhs = Wv chunk [128(E), 640].
        v_ps0 = psum.tile([B, 512], f32, tag="ps")
        v_ps1 = psum.tile([B, EPC - 512], f32, tag="ps")
        for g in range(10):
            wt = wpool.tile([128, 4 * EPC], f32, tag="w")
            nc.gpsimd.dma_start(
                out=wt[:],
                in_=qkvw[2, g * 512:(g + 1) * 512, :].rearrange(
                    "(kl p) c -> p kl c", p=128
                ),
            )
            for kl in range(4):
                kc = g * 4 + kl
                nc.tensor.matmul(
                    v_ps0[:],
                    lhsT=hT_sb[:, kc * B:(kc + 1) * B],
                    rhs=wt[:, kl * EPC: kl * EPC + 512],
                    start=(kc == 0),
                    stop=(kc == 39),
                )
                nc.tensor.matmul(
                    v_ps1[:],
                    lhsT=hT_sb[:, kc * B:(kc + 1) * B],
                    rhs=wt[:, kl * EPC + 512: kl * EPC + EPC],
                    start=(kc == 0),
                    stop=(kc == 39),
                )
        nc.scalar.copy(v_sb[:, :512], v_ps0[:])
        nc.scalar.copy(v_sb[:, 512:], v_ps1[:])

        # ---- attention per (b, h) ----
        for b in range(B):
            n = nch[b]
            sd = n * 128
            p = pos[b]
            for h in range(HPC):
                col = h * B + b
                Kt = kvpool.tile([128, S], f32, tag="K")
                nc.gpsimd.dma_start(out=Kt[:, :sd], in_=kt[h, b, :, :sd])
                Vt = kvpool.tile([128, 16, D], f32, tag="V")
                nc.gpsimd.dma_start(out=Vt[:, :n, :], in_=vt[h, b, :, :n, :])

                # scatter the new token K column (same partitions: d)
                nc.vector.tensor_copy(Kt[:, p:p + 1], kT_sb[:, col:col + 1])
                # scatter the new token V row (cross-partition -> DMA)
                nc.gpsimd.dma_start(
                    out=Vt[p % 128:p % 128 + 1, p // 128, :],
                    in_=v_sb[b:b + 1, h * D:(h + 1) * D],
                )

                sc_ps = psum.tile([128, 16], f32, tag="ps")
                for c in range(n):
                    nc.tensor.matmul(
                        sc_ps[:, c:c + 1],
                        lhsT=Kt[:, c * 128:(c + 1) * 128],
                        rhs=qT_sb[:, col:col + 1],
                        start=True,
                        stop=True,
                    )
                s_sb = tmp.tile([128, 16], f32, tag="s")
                nc.vector.tensor_add(
                    s_sb[:, :n],
                    sc_ps[:, :n],
                    bias_sb[:, (b * HPC + h) * 16:(b * HPC + h) * 16 + n],
                )
                attn_sb = tmp.tile([128, 16], f32, tag="attn")
                nc.scalar.activation(
                    attn_sb[:, :n],
                    s_sb[:, :n],
                    func=mybir.ActivationFunctionType.Exp,
                    accum_out=colsum_sb[:, col:col + 1],
                )
                ao_ps = psum.tile([128, 1], f32, tag="ps")
                for c in range(n):
                    nc.tensor.matmul(
                        ao_ps[:],
                        lhsT=Vt[:, c, :],
                        rhs=attn_sb[:, c:c + 1],
                        start=(c == 0),
                        stop=(c == n - 1),
                    )
                nc.scalar.copy(aoT_sb[:, col:col + 1], ao_ps[:])

        # ---- softmax normalization (batched over all 20 (b,h)) ----
        sums_ps = psum.tile([1, HPC * B], f32, tag="ps")
        nc.tensor.matmul(
            sums_ps[:], lhsT=ones_col[:], rhs=colsum_sb[:], start=True, stop=True
        )
        recip_sb = tmp.tile([1, HPC * B], f32, tag="recip")
        nc.vector.reciprocal(recip_sb[:], sums_ps[:])
        rb_ps = psum.tile([128, HPC * B], f32, tag="ps")
        nc.tensor.matmul(
            rb_ps[:], lhsT=ones_row[:], rhs=recip_sb[:], start=True, stop=True
        )
        recip_b = tmp.tile([128, HPC * B], f32, tag="recipb")
        nc.vector.tensor_copy(recip_b[:], rb_ps[:])
        attn_nT = consts.tile([128, HPC * B], f32)
        nc.vector.tensor_mul(attn_nT[:], aoT_sb[:], recip_b[:])

        # ---- output projection (transposed): outT[oc*128+p, b] ----
        # lhsT = o chunk [128(hd), 128(oc)], rhs = attn_nT slice [128(hd), B]
        for jg in range(5):  # groups of 8 outcol chunks (1024 cols)
            ops = [psum.tile([128, B], f32, tag="ps", name=f"ps_o{jg}_{i}") for i in range(8)]
            for h in range(HPC):
                ot = opool.tile([128, 1024], f32, tag="ot")
                nc.gpsimd.dma_start(
                    out=ot[:],
                    in_=ow[h * 128:(h + 1) * 128, jg * 1024:(jg + 1) * 1024],
                )
                for oc in range(8):
                    nc.tensor.matmul(
                        ops[oc][:],
                        lhsT=ot[:, oc * 128:(oc + 1) * 128],
                        rhs=attn_nT[:, h * B:(h + 1) * B],
                        start=(h == 0),
                        stop=(h == HPC - 1),
                    )
            for oc in range(8):
                g_oc = jg * 8 + oc
                nc.scalar.copy(outT_sb[:, g_oc * B:(g_oc + 1) * B], ops[oc][:])

        nc.gpsimd.dma_start(out=outT[:], in_=outT_sb[:])

    nc.compile()  # Bacc finalize: splits multi-waits (matmul 1-wait limit)
    return nc


def _prepare_core_inputs(core, hidden, qkv_w, o_w, k_cache, v_cache, bt, sl, pos):
    hs = slice(core * HPC, (core + 1) * HPC)
    es = slice(core * EPC, (core + 1) * EPC)

    qkvw = np.ascontiguousarray(qkv_w[:, :, es])
    qkvw[0] *= np.float32(D ** -0.5)

    kg = k_cache[:, hs]  # [NB, HPC, BS, D]
    vg = v_cache[:, hs]
    kt = np.empty((HPC, B, D, S), np.float32)
    vt = np.empty((HPC, B, 128, 16, D), np.float32)
    for b in range(B):
        kk = kg[bt[b]].transpose(1, 0, 2, 3).reshape(HPC, S, D)
        kt[:, b] = kk.transpose(0, 2, 1)
        vv = vg[bt[b]].transpose(1, 0, 2, 3).reshape(HPC, S, D)
        vt[:, b] = vv.reshape(HPC, 16, 128, D).transpose(0, 2, 1, 3)

    slopes = _alibi_slopes(H)[core * HPC:(core + 1) * HPC]
    t_in = np.arange(128)[:, None]
    tg = (np.arange(16)[None, :] * 128 + t_in).astype(np.float32)  # [128, 16]
    bias = np.empty((128, B, HPC, 16), np.float32)
    for b in range(B):
        for h in range(HPC):
            val = slopes[h] * (tg - np.float32(pos[b]))
            val[tg >= sl[b]] = NEG
            bias[:, b, h, :] = val

    hTf = np.ascontiguousarray(
        hidden.T.reshape(40, 128, B).transpose(1, 0, 2).reshape(128, 40 * B)
    )

    return dict(
        hT=hTf,
        qkvw=qkvw,
        ow=np.ascontiguousarray(o_w[es, :]),
        kt=kt,
        vt=vt,
        bias=np.ascontiguousarray(bias.reshape(128, B * HPC * 16)),
    )


def kernel(**inputs):
    global LAST_RESULTS
    hidden = np.asarray(inputs["hidden_states"], np.float32)
    qkv_w = np.asarray(inputs["qkv_weight"], np.float32)
    o_w = np.asarray(inputs["o_proj_weight"], np.float32)
    k_cache = np.asarray(inputs["k_cache"], np.float32)
    v_cache = np.asarray(inputs["v_cache"], np.float32)
    bt = np.asarray(inputs["block_tables"]).astype(np.int64)
    sl = np.asarray(inputs["sequence_lengths"]).astype(np.int64)

    pos = tuple(int(x) - 1 for x in sl)
    nch = tuple(int(math.ceil(int(x) / 128)) for x in sl)

    in_maps = [
        _prepare_core_inputs(c, hidden, qkv_w, o_w, k_cache, v_cache, bt, sl, pos)
        for c in range(NCORES)
    ]

    key = (pos, nch)
    if key not in _PROGRAM_CACHE:
        _PROGRAM_CACHE[key] = _build_program(pos, nch)
    nc = _PROGRAM_CACHE[key]

    from concourse.bass_utils import run_bass_kernel_spmd

    res = run_bass_kernel_spmd(
        nc,
        in_maps,
        core_ids=list(range(NCORES)),
        trace=bool(os.environ.get("BASS_TRACE")),
    )
    LAST_RESULTS = res

    out = np.zeros((B, E), np.float64)
    for c in range(NCORES):
        r = np.asarray(res.results[c]["outT"])
        out += r.reshape(128, 40, B).transpose(2, 1, 0).reshape(B, E).astype(np.float64)
    return out.astype(np.float32)



# revision 2
# speedup vs baseline: 3.1766x; 3.1766x over previous
"""Paged KV-cache decode attention with ALiBi (Baichuan-style), fused
QKV + attention + output projection, tensor-parallel over heads across
8 Trainium2 NeuronCores.

v2: fp16 everywhere (weights, K/V cache, activations) — the fp32
baseline was tensor-engine bound (fp32 matmuls run as 2 HI/LO passes,
no fast-weight-load). fp16 halves HBM traffic AND runs matmuls at full
rate with FWL, making the kernel DMA-roofline bound (~41 MB/core).

Layout strategy (per core, 5 heads):
  - qT/kT computed as [640, 4] (head-dim on partitions) so scores matmuls
    need no transposes and the K-cache new-token scatter is a same-partition
    SBUF copy.
  - v computed as [4, 640] (natural) so the V new-token scatter is a tiny
    SBUF->SBUF DMA row write.
  - K cache packed host-side per core as [5, 128(d), sum_sd] (K^T, only
    the used chunks, concatenated across the 4 seqs), V cache as
    [5, 128(t%128), sum_nb, 128(d)] — ONE large (~1.4MB) DMA per head.
  - Weight DMAs (qkvw/ow) go on the gpsimd (SWDGE) queue; K/V/ow bulk
    loads go on the sync (HWDGE) queue so buffer-reuse waits in one
    stream never stall the other and HBM stays saturated.
  - softmax without max-subtraction (scores are O(10); exp is safe),
    masking baked into a host-precomputed additive fp32 bias (-1e30).
  - o_proj computed transposed (out^T [5120, 4]) per core; host sums the
    8 partial products (the "all-reduce").
"""

import math
import os
import sys
from contextlib import ExitStack

import numpy as np

sys.path.insert(0, "/opt/trn_rl_repo")

B = 4
E = 5120
H = 40
D = 128
BS = 16
NB = 512
MB = 128
S = MB * BS  # 2048
NCORES = 8
HPC = H // NCORES   # 5 heads per core
EPC = HPC * D       # 640

NEG = -1.0e30
GK = 10             # E-chunks (of 128) per qkv weight DMA group


def _alibi_slopes(num_heads):
    cp2 = 2 ** int(math.floor(math.log2(num_heads)))
    base = 2.0 ** (-(2.0 ** (-(math.log2(cp2) - 3))))
    slopes = base ** np.arange(1, cp2 + 1, dtype=np.float64)
    if cp2 != num_heads:
        extra_base = 2.0 ** (-(2.0 ** (-(math.log2(2 * cp2) - 3))))
        n_rem = min(cp2, num_heads - cp2)
        extra = extra_base ** np.arange(1, 1 + 2 * n_rem, 2, dtype=np.float64)
        slopes = np.concatenate([slopes, extra])
    return slopes.astype(np.float32)


_PROGRAM_CACHE = {}
LAST_RESULTS = None  # BassKernelResults of the most recent run (for test.py)


def _build_program(pos, nch):
    """Build the SPMD Bass program. pos/nch are per-sequence tuples, baked
    statically (same for all cores; per-core data varies only via inputs)."""
    import concourse.bacc as bacc
    import concourse.bass as bass
    import concourse.tile as tile
    from concourse import mybir

    f32 = mybir.dt.float32
    f16 = mybir.dt.float16
    nc = bacc.Bacc()

    sumnb = sum(nch)
    sumsd = sumnb * 128
    offc = [0]
    for n in nch:
        offc.append(offc[-1] + n)

    hT = nc.declare_dram_parameter("hT", [128, 40 * B], f16, isOutput=False)
    qkvw = nc.declare_dram_parameter("qkvw", [3, E, EPC], f16, isOutput=False)
    ow = nc.declare_dram_parameter("ow", [128, HPC * E], f16, isOutput=False)
    kt = nc.declare_dram_parameter("kt", [HPC, D, sumsd], f16, isOutput=False)
    vt = nc.declare_dram_parameter("vt", [HPC, 128, sumnb, D], f16, isOutput=False)
    bias = nc.declare_dram_parameter("bias", [128, B * HPC * 16], f32, isOutput=False)
    outT = nc.declare_dram_parameter("outT", [128, 40 * B], f32, isOutput=True)

    with tile.TileContext(nc) as tc, ExitStack() as ctx:
        consts = ctx.enter_context(tc.tile_pool(name="consts", bufs=1))
        wpool = ctx.enter_context(tc.tile_pool(name="wpool", bufs=3))
        kpool = ctx.enter_context(tc.tile_pool(name="kpool", bufs=3))
        vpool = ctx.enter_context(tc.tile_pool(name="vpool", bufs=3))
        tmp = ctx.enter_context(tc.tile_pool(name="tmp", bufs=4))
        psum = ctx.enter_context(tc.tile_pool(name="psum", bufs=8, space="PSUM"))

        # ---- no-dep bulk loads issued up-front on the sync (HWDGE) queue ----
        ow_sb = consts.tile([128, HPC * E], f16)
        nc.sync.dma_start(out=ow_sb[:, : HPC * E // 2], in_=ow[:, : HPC * E // 2])
        nc.sync.dma_start(out=ow_sb[:, HPC * E // 2:], in_=ow[:, HPC * E // 2:])

        # ---- constants / small inputs (gpsimd/SWDGE queue) ----
        hT_sb = consts.tile([128, 40 * B], f16)          # (E%128, (Echunk, b))
        nc.gpsimd.dma_start(out=hT_sb[:], in_=hT[:])
        bias_sb = consts.tile([128, B * HPC * 16], f32)  # (t%128, (b, h, chunk))
        nc.gpsimd.dma_start(out=bias_sb[:], in_=bias[:])
        ones_col = consts.tile([128, 1], f32)
        nc.vector.memset(ones_col[:], 1.0)
        ones_row = consts.tile([1, 128], f32)
        nc.vector.memset(ones_row[:], 1.0)

        qT_sb = consts.tile([128, HPC * B], f16)   # col = h*B + b ; partition = d
        kT_sb = consts.tile([128, HPC * B], f16)
        v_sb = consts.tile([B, EPC], f16)          # natural v rows
        colsum_sb = consts.tile([128, HPC * B], f32)
        aoT_sb = consts.tile([128, HPC * B], f32)  # unnormalized attn@V ^T
        outT_sb = consts.tile([128, 40 * B], f32)

        # ---- fused QKV projection ----
        # q,k transposed orientation: psum[oc] [128, B] accumulated over 40
        # E-chunks; lhsT = W chunk [128(E), 128(outcol)], rhs = hT chunk [128(E), B].
        for w in range(2):  # 0=q (pre-scaled on host), 1=k
            dst = qT_sb if w == 0 else kT_sb
            ps = [psum.tile([128, B], f32, tag="ps", name=f"ps_qk{w}_{i}") for i in range(HPC)]
            for g in range(40 // GK):
                wt = wpool.tile([128, GK * EPC], f16, tag="w")
                nc.gpsimd.dma_start(
                    out=wt[:],
                    in_=qkvw[w, g * GK * 128:(g + 1) * GK * 128, :].rearrange(
                        "(kl p) c -> p kl c", p=128
                    ),
                )
                for oc in range(HPC):
                    for kl in range(GK):
                        kc = g * GK + kl
                        nc.tensor.matmul(
                            ps[oc][:],
                            lhsT=wt[:, kl * EPC + oc * 128: kl * EPC + (oc + 1) * 128],
                            rhs=hT_sb[:, kc * B:(kc + 1) * B],
                            start=(kc == 0),
                            stop=(kc == 39),
                        )
            for oc in range(HPC):
                nc.scalar.copy(dst[:, oc * B:(oc + 1) * B], ps[oc][:])

        # v natural orientation: psum [B, 640] (two banks: 512 + 128),
        # lhsT = hT chunk [128(E), B], rhs = Wv chunk [128(E), 640].
        v_ps0 = psum.tile([B, 512], f32, tag="ps")
        v_ps1 = psum.tile([B, EPC - 512], f32, tag="ps")
        for g in range(40 // GK):
            wt = wpool.tile([128, GK * EPC], f16, tag="w")
            nc.gpsimd.dma_start(
                out=wt[:],
                in_=qkvw[2, g * GK * 128:(g + 1) * GK * 128, :].rearrange(
                    "(kl p) c -> p kl c", p=128
                ),
            )
            for kl in range(GK):
                kc = g * GK + kl
                nc.tensor.matmul(
                    v_ps0[:],
                    lhsT=hT_sb[:, kc * B:(kc + 1) * B],
                    rhs=wt[:, kl * EPC: kl * EPC + 512],
                    start=(kc == 0),
                    stop=(kc == 39),
                )
                nc.tensor.matmul(
                    v_ps1[:],
                    lhsT=hT_sb[:, kc * B:(kc + 1) * B],
                    rhs=wt[:, kl * EPC + 512: kl * EPC + EPC],
                    start=(kc == 0),
                    stop=(kc == 39),
                )
        nc.scalar.copy(v_sb[:, :512], v_ps0[:])
        nc.scalar.copy(v_sb[:, 512:], v_ps1[:])

        # ---- attention, head-major so per-head K/V tiles stream ----
        for h in range(HPC):
            Kt = kpool.tile([128, sumsd], f16, tag="K")
            nc.sync.dma_start(out=Kt[:], in_=kt[h])
            Vt = vpool.tile([128, sumnb, D], f16, tag="V")
            nc.sync.dma_start(out=Vt[:], in_=vt[h])
            for b in range(B):
                col = h * B + b
                p = pos[b]
                n = nch[b]
                off = offc[b]

                # scatter the new token K column (same partitions: d)
                nc.vector.tensor_copy(
                    Kt[:, off * 128 + p: off * 128 + p + 1], kT_sb[:, col:col + 1]
                )
                # scatter the new token V row (cross-partition -> DMA)
                nc.gpsimd.dma_start(
                    out=Vt[p % 128: p % 128 + 1, off + p // 128, :],
                    in_=v_sb[b:b + 1, h * D:(h + 1) * D],
                )

                sc_ps = psum.tile([128, 16], f32, tag="ps")
                for c in range(n):
                    nc.tensor.matmul(
                        sc_ps[:, c:c + 1],
                        lhsT=Kt[:, (off + c) * 128:(off + c + 1) * 128],
                        rhs=qT_sb[:, col:col + 1],
                        start=True,
                        stop=True,
                    )
                s_sb = tmp.tile([128, 16], f32, tag="s")
                nc.vector.tensor_add(
                    s_sb[:, :n],
                    sc_ps[:, :n],
                    bias_sb[:, (b * HPC + h) * 16:(b * HPC + h) * 16 + n],
                )
                attn_sb = tmp.tile([128, 16], f16, tag="attn")
                nc.scalar.activation(
                    attn_sb[:, :n],
                    s_sb[:, :n],
                    func=mybir.ActivationFunctionType.Exp,
                    accum_out=colsum_sb[:, col:col + 1],
                )
                ao_ps = psum.tile([128, 1], f32, tag="ps")
                for c in range(n):
                    nc.tensor.matmul(
                        ao_ps[:],
                        lhsT=Vt[:, off + c, :],
                        rhs=attn_sb[:, c:c + 1],
                        start=(c == 0),
                        stop=(c == n - 1),
                    )
                nc.scalar.copy(aoT_sb[:, col:col + 1], ao_ps[:])

        # ---- softmax normalization (batched over all 20 (b,h)) ----
        sums_ps = psum.tile([1, HPC * B], f32, tag="ps")
        nc.tensor.matmul(
            sums_ps[:], lhsT=ones_col[:], rhs=colsum_sb[:], start=True, stop=True
        )
        recip_sb = tmp.tile([1, HPC * B], f32, tag="recip")
        nc.vector.reciprocal(recip_sb[:], sums_ps[:])
        rb_ps = psum.tile([128, HPC * B], f32, tag="ps")
        nc.tensor.matmul(
            rb_ps[:], lhsT=ones_row[:], rhs=recip_sb[:], start=True, stop=True
        )
        recip_b = tmp.tile([128, HPC * B], f32, tag="recipb")
        nc.vector.tensor_copy(recip_b[:], rb_ps[:])
        attn_nT = consts.tile([128, HPC * B], f16)
        nc.vector.tensor_mul(attn_nT[:], aoT_sb[:], recip_b[:])

        # ---- output projection (transposed): outT[oc*128+p, b] ----
        # lhsT = ow chunk [128(hd%128), 128(outcol)], rhs = attn_nT slice [128(hd), B]
        for jg in range(5):  # groups of 8 outcol chunks (1024 cols)
            ops = [psum.tile([128, B], f32, tag="ps", name=f"ps_o{jg}_{i}") for i in range(8)]
            for hh in range(HPC):
                for oc in range(8):
                    j0 = jg * 1024 + oc * 128
                    nc.tensor.matmul(
                        ops[oc][:],
                        lhsT=ow_sb[:, hh * E + j0: hh * E + j0 + 128],
                        rhs=attn_nT[:, hh * B:(hh + 1) * B],
                        start=(hh == 0),
                        stop=(hh == HPC - 1),
                    )
            for oc in range(8):
                g_oc = jg * 8 + oc
                nc.scalar.copy(outT_sb[:, g_oc * B:(g_oc + 1) * B], ops[oc][:])

        nc.gpsimd.dma_start(out=outT[:], in_=outT_sb[:])

    nc.compile()  # Bacc finalize: splits multi-waits (matmul 1-wait limit)
    return nc


def _prepare_core_inputs(core, hidden16, qkv16, o16, k16, v16, bt, sl, pos, nch):
    """Per-core staged arrays. hidden16/qkv16/o16/k16/v16 are fp16 full
    tensors (cast once by kernel()); this slices + packs layouts."""
    hs = slice(core * HPC, (core + 1) * HPC)
    es = slice(core * EPC, (core + 1) * EPC)

    qkvw = np.ascontiguousarray(qkv16[:, :, es])

    sumnb = sum(nch)
    sumsd = sumnb * 128
    offc = [0]
    for n in nch:
        offc.append(offc[-1] + n)

    kg = k16[:, hs]  # [NB, HPC, BS, D]
    vg = v16[:, hs]
    kt = np.zeros((HPC, D, sumsd), np.float16)
    vt = np.zeros((HPC, 128, sumnb, D), np.float16)
    for b in range(B):
        sd = nch[b] * 128
        blocks = bt[b][: (sd + BS - 1) // BS]  # blocks covering the used chunks
        kk = kg[blocks].transpose(1, 0, 2, 3).reshape(HPC, sd, D)
        kt[:, :, offc[b] * 128: offc[b] * 128 + sd] = kk.transpose(0, 2, 1)
        vv = vg[blocks].transpose(1, 0, 2, 3).reshape(HPC, sd, D)
        vt[:, :, offc[b]: offc[b] + nch[b], :] = vv.reshape(
            HPC, nch[b], 128, D
        ).transpose(0, 2, 1, 3)

    slopes = _alibi_slopes(H)[core * HPC:(core + 1) * HPC]
    t_in = np.arange(128)[:, None]
    tg = (np.arange(16)[None, :] * 128 + t_in).astype(np.float32)  # [128, 16]
    biasa = np.empty((128, B, HPC, 16), np.float32)
    for b in range(B):
        for h in range(HPC):
            val = slopes[h] * (tg - np.float32(pos[b]))
            val[tg >= sl[b]] = NEG
            biasa[:, b, h, :] = val

    hTf = np.ascontiguousarray(
        hidden16.T.reshape(40, 128, B).transpose(1, 0, 2).reshape(128, 40 * B)
    )

    # ow pre-transposed: owr[p, h*E + j] = o_proj_weight[core*EPC + h*128 + p, j]
    owr = np.ascontiguousarray(
        o16[es].reshape(HPC, 128, E).transpose(1, 0, 2).reshape(128, HPC * E)
    )

    return dict(
        hT=hTf,
        qkvw=qkvw,
        ow=owr,
        kt=kt,
        vt=vt,
        bias=np.ascontiguousarray(biasa.reshape(128, B * HPC * 16)),
    )


def kernel(**inputs):
    global LAST_RESULTS
    hidden = np.asarray(inputs["hidden_states"], np.float32)
    qkv_w = np.asarray(inputs["qkv_weight"], np.float32)
    o_w = np.asarray(inputs["o_proj_weight"], np.float32)
    k_cache = np.asarray(inputs["k_cache"], np.float32)
    v_cache = np.asarray(inputs["v_cache"], np.float32)
    bt = np.asarray(inputs["block_tables"]).astype(np.int64)
    sl = np.asarray(inputs["sequence_lengths"]).astype(np.int64)

    pos = tuple(int(x) - 1 for x in sl)
    nch = tuple(int(math.ceil(int(x) / 128)) for x in sl)

    # cast once to fp16 (q pre-scaled by 1/sqrt(D) before the cast)
    hidden16 = hidden.astype(np.float16)
    qkv16 = qkv_w.copy()
    qkv16[0] *= np.float32(D ** -0.5)
    qkv16 = qkv16.astype(np.float16)
    o16 = o_w.astype(np.float16)
    k16 = k_cache.astype(np.float16)
    v16 = v_cache.astype(np.float16)

    in_maps = [
        _prepare_core_inputs(c, hidden16, qkv16, o16, k16, v16, bt, sl, pos, nch)
        for c in range(NCORES)
    ]

    key = (pos, nch)
    if key not in _PROGRAM_CACHE:
        _PROGRAM_CACHE[key] = _build_program(pos, nch)
    nc = _PROGRAM_CACHE[key]

    from concourse.bass_utils import run_bass_kernel_spmd

    res = run_bass_kernel_spmd(
        nc,
        in_maps,
        core_ids=list(range(NCORES)),
        trace=bool(os.environ.get("BASS_TRACE")),
    )
    LAST_RESULTS = res

    out = np.zeros((B, E), np.float64)
    for c in range(NCORES):
        r = np.asarray(res.results[c]["outT"])
        out += r.reshape(128, 40, B).transpose(2, 1, 0).reshape(B, E).astype(np.float64)
    return out.astype(np.float32)


# revision 7
# speedup vs baseline: 3.6732x; 1.1564x over previous
"""Paged KV-cache decode attention with ALiBi (Baichuan-style), fused
QKV + attention + output projection, tensor-parallel over heads across
8 Trainium2 NeuronCores.

v2: fp16 everywhere (weights, K/V cache, activations) — the fp32
baseline was tensor-engine bound (fp32 matmuls run as 2 HI/LO passes,
no fast-weight-load). fp16 halves HBM traffic AND runs matmuls at full
rate with FWL, making the kernel DMA-roofline bound (~41 MB/core).

Layout strategy (per core, 5 heads):
  - qT/kT computed as [640, 4] (head-dim on partitions) so scores matmuls
    need no transposes and the K-cache new-token scatter is a same-partition
    SBUF copy.
  - v computed as [4, 640] (natural) so the V new-token scatter is a tiny
    SBUF->SBUF DMA row write.
  - K cache packed host-side per core as [5, 128(d), sum_sd] (K^T, only
    the used chunks, concatenated across the 4 seqs), V cache as
    [5, 128(t%128), sum_nb, 128(d)] — ONE large (~1.4MB) DMA per head.
  - Weight DMAs (qkvw/ow) go on the gpsimd (SWDGE) queue; K/V/ow bulk
    loads go on the sync (HWDGE) queue so buffer-reuse waits in one
    stream never stall the other and HBM stays saturated.
  - softmax without max-subtraction (scores are O(10); exp is safe),
    masking baked into a host-precomputed additive fp32 bias (-1e30).
  - o_proj computed transposed (out^T [5120, 4]) per core; host sums the
    8 partial products (the "all-reduce").
"""

import math
import os
import sys
from contextlib import ExitStack

import numpy as np

sys.path.insert(0, "/opt/trn_rl_repo")

B = 4
E = 5120
H = 40
D = 128
BS = 16
NB = 512
MB = 128
S = MB * BS  # 2048
NCORES = 8
HPC = H // NCORES   # 5 heads per core
EPC = HPC * D       # 640

NEG = -1.0e30
GK = 10             # E-chunks (of 128) per qkv weight DMA group


def _alibi_slopes(num_heads):
    cp2 = 2 ** int(math.floor(math.log2(num_heads)))
    base = 2.0 ** (-(2.0 ** (-(math.log2(cp2) - 3))))
    slopes = base ** np.arange(1, cp2 + 1, dtype=np.float64)
    if cp2 != num_heads:
        extra_base = 2.0 ** (-(2.0 ** (-(math.log2(2 * cp2) - 3))))
        n_rem = min(cp2, num_heads - cp2)
        extra = extra_base ** np.arange(1, 1 + 2 * n_rem, 2, dtype=np.float64)
        slopes = np.concatenate([slopes, extra])
    return slopes.astype(np.float32)


_PROGRAM_CACHE = {}
LAST_RESULTS = None  # BassKernelResults of the most recent run (for test.py)


def _build_program(pos, nch):
    """Build the SPMD Bass program. pos/nch are per-sequence tuples, baked
    statically (same for all cores; per-core data varies only via inputs)."""
    import concourse.bacc as bacc
    import concourse.bass as bass
    import concourse.tile as tile
    from concourse import mybir

    f32 = mybir.dt.float32
    f16 = mybir.dt.float16
    nc = bacc.Bacc()

    sumnb = sum(nch)
    sumsd = sumnb * 128
    offc = [0]
    for n in nch:
        offc.append(offc[-1] + n)

    hT = nc.declare_dram_parameter("hT", [128, 40 * B], f16, isOutput=False)
    qkvw = nc.declare_dram_parameter("qkvw", [3, 128, 40, EPC], f16, isOutput=False)
    ow = nc.declare_dram_parameter("ow", [128, HPC * E], f16, isOutput=False)
    kt = nc.declare_dram_parameter("kt", [HPC, D, sumsd], f16, isOutput=False)
    vt = nc.declare_dram_parameter("vt", [HPC, 128, sumnb, D], f16, isOutput=False)
    bias = nc.declare_dram_parameter("bias", [128, B * HPC * 16], f32, isOutput=False)
    outT = nc.declare_dram_parameter("outT", [128, 40 * B], f32, isOutput=True)

    NG = 40 // GK  # weight DMA groups per tensor

    with tile.TileContext(nc) as tc, ExitStack() as ctx:
        consts = ctx.enter_context(tc.tile_pool(name="consts", bufs=1))
        wpool = ctx.enter_context(tc.tile_pool(name="wpool", bufs=4))
        kpool = ctx.enter_context(tc.tile_pool(name="kpool", bufs=3))
        vpool = ctx.enter_context(tc.tile_pool(name="vpool", bufs=3))
        tmp = ctx.enter_context(tc.tile_pool(name="tmp", bufs=4))
        psum = ctx.enter_context(tc.tile_pool(name="psum", bufs=8, space="PSUM"))

        hT_sb = consts.tile([128, 40 * B], f16)          # (E%128, (Echunk, b))
        bias_sb = consts.tile([128, B * HPC * 16], f32)  # (t%128, (b, h, chunk))
        ow_sb = consts.tile([128, HPC * E], f16)
        qT_sb = consts.tile([128, HPC * B], f16)   # col = h*B + b ; partition = d
        kT_sb = consts.tile([128, HPC * B], f16)
        v_sb = consts.tile([B, EPC], f16)          # natural v rows
        colsum_sb = consts.tile([128, HPC * B], f32)
        aoT_sb = consts.tile([128, HPC * B], f32)  # unnormalized attn@V ^T
        outT_sb = consts.tile([128, 40 * B], f32)

        ones_col = consts.tile([128, 1], f32)
        nc.vector.memset(ones_col[:], 1.0)
        ones_row = consts.tile([1, 128], f32)
        nc.vector.memset(ones_row[:], 1.0)

        # ---- the bulk DMA stream: ONE queue (gpsimd/SWDGE), explicitly
        # ordered so HBM never starves and each tile lands just before its
        # consumer needs it. Buffer-reuse waits stall only the issue front,
        # never the SDMA backlog (pools sized so waits resolve early).
        wq, wk, wv = [], [], []
        Kts = [None] * HPC
        Vts = [None] * HPC

        def qkv_group(w, lst):
            t = wpool.tile([128, GK, EPC], f16, tag="w", name=f"w{w}_{len(lst)}")
            nc.gpsimd.dma_start(out=t[:], in_=qkvw[w, :, len(lst) * GK:(len(lst) + 1) * GK, :])
            lst.append(t)

        def load_k(h):
            Kts[h] = kpool.tile([128, sumsd], f16, tag="K", name=f"K{h}")
            nc.gpsimd.dma_start(out=Kts[h][:], in_=kt[h])

        def load_v(h):
            Vts[h] = vpool.tile([128, sumnb, D], f16, tag="V", name=f"V{h}")
            nc.gpsimd.dma_start(out=Vts[h][:], in_=vt[h])

        nc.gpsimd.dma_start(out=hT_sb[:], in_=hT[:])
        nc.gpsimd.dma_start(out=bias_sb[:], in_=bias[:])
        for g in range(NG):
            qkv_group(0, wq)
        load_k(0)
        for g in range(NG):
            qkv_group(1, wk)
        load_k(1)
        load_v(0)
        for g in range(NG):
            qkv_group(2, wv)
        load_v(1)
        load_k(2)
        load_v(2)
        load_k(3)
        load_v(3)
        load_k(4)
        load_v(4)
        nc.gpsimd.dma_start(out=ow_sb[:, : HPC * E // 2], in_=ow[:, : HPC * E // 2])
        nc.gpsimd.dma_start(out=ow_sb[:, HPC * E // 2:], in_=ow[:, HPC * E // 2:])

        # ---- fused QKV projection ----
        # q,k transposed orientation: psum[oc] [128, B] accumulated over 40
        # E-chunks; lhsT = W chunk [128(E), 128(outcol)], rhs = hT chunk [128(E), B].
        for w, lst in ((0, wq), (1, wk)):  # 0=q (pre-scaled on host), 1=k
            dst = qT_sb if w == 0 else kT_sb
            ps = [psum.tile([128, B], f32, tag="ps", name=f"ps_qk{w}_{i}") for i in range(HPC)]
            for g in range(NG):
                wt = lst[g]
                for oc in range(HPC):
                    for kl in range(GK):
                        kc = g * GK + kl
                        nc.tensor.matmul(
                            ps[oc][:],
                            lhsT=wt[:, kl, oc * 128:(oc + 1) * 128],
                            rhs=hT_sb[:, kc * B:(kc + 1) * B],
                            start=(kc == 0),
                            stop=(kc == 39),
                        )
            for oc in range(HPC):
                nc.scalar.copy(dst[:, oc * B:(oc + 1) * B], ps[oc][:])

        # v natural orientation: psum [B, 640] (two banks: 512 + 128),
        # lhsT = hT chunk [128(E), B], rhs = Wv chunk [128(E), 640].
        v_ps0 = psum.tile([B, 512], f32, tag="ps")
        v_ps1 = psum.tile([B, EPC - 512], f32, tag="ps")
        for g in range(NG):
            wt = wv[g]
            for kl in range(GK):
                kc = g * GK + kl
                nc.tensor.matmul(
                    v_ps0[:],
                    lhsT=hT_sb[:, kc * B:(kc + 1) * B],
                    rhs=wt[:, kl, :512],
                    start=(kc == 0),
                    stop=(kc == 39),
                )
                nc.tensor.matmul(
                    v_ps1[:],
                    lhsT=hT_sb[:, kc * B:(kc + 1) * B],
                    rhs=wt[:, kl, 512:],
                    start=(kc == 0),
                    stop=(kc == 39),
                )
        nc.scalar.copy(v_sb[:, :512], v_ps0[:])
        nc.scalar.copy(v_sb[:, 512:], v_ps1[:])

        # ---- attention, head-major so per-head K/V tiles stream ----
        for h in range(HPC):
            Kt = Kts[h]
            Vt = Vts[h]
            for b in range(B):
                col = h * B + b
                p = pos[b]
                n = nch[b]
                off = offc[b]

                # scatter the new token K column (same partitions: d)
                nc.vector.tensor_copy(
                    Kt[:, off * 128 + p: off * 128 + p + 1], kT_sb[:, col:col + 1]
                )
                # scatter the new token V row (cross-partition -> DMA).
                # HWDGE via the scalar engine: keeps it off the bulk
                # (gpsimd) stream, and the preceding v_sb copy runs on this
                # same queue so the issue never stalls the exp chain.
                nc.scalar.dma_start(
                    out=Vt[p % 128: p % 128 + 1, off + p // 128, :],
                    in_=v_sb[b:b + 1, h * D:(h + 1) * D],
                )

                sc_ps = psum.tile([128, 16], f32, tag="ps")
                for c in range(n):
                    nc.tensor.matmul(
                        sc_ps[:, c:c + 1],
                        lhsT=Kt[:, (off + c) * 128:(off + c + 1) * 128],
                        rhs=qT_sb[:, col:col + 1],
                        start=True,
                        stop=True,
                    )
                s_sb = tmp.tile([128, 16], f32, tag="s")
                nc.vector.tensor_add(
                    s_sb[:, :n],
                    sc_ps[:, :n],
                    bias_sb[:, (b * HPC + h) * 16:(b * HPC + h) * 16 + n],
                )
                attn_sb = tmp.tile([128, 16], f16, tag="attn")
                nc.scalar.activation(
                    attn_sb[:, :n],
                    s_sb[:, :n],
                    func=mybir.ActivationFunctionType.Exp,
                    accum_out=colsum_sb[:, col:col + 1],
                )
                ao_ps = psum.tile([128, 1], f32, tag="ps")
                for c in range(n):
                    nc.tensor.matmul(
                        ao_ps[:],
                        lhsT=Vt[:, off + c, :],
                        rhs=attn_sb[:, c:c + 1],
                        start=(c == 0),
                        stop=(c == n - 1),
                    )
                nc.scalar.copy(aoT_sb[:, col:col + 1], ao_ps[:])

        # ---- softmax normalization (batched over all 20 (b,h)) ----
        sums_ps = psum.tile([1, HPC * B], f32, tag="ps")
        nc.tensor.matmul(
            sums_ps[:], lhsT=ones_col[:], rhs=colsum_sb[:], start=True, stop=True
        )
        recip_sb = tmp.tile([1, HPC * B], f32, tag="recip")
        nc.vector.reciprocal(recip_sb[:], sums_ps[:])
        rb_ps = psum.tile([128, HPC * B], f32, tag="ps")
        nc.tensor.matmul(
            rb_ps[:], lhsT=ones_row[:], rhs=recip_sb[:], start=True, stop=True
        )
        recip_b = tmp.tile([128, HPC * B], f32, tag="recipb")
        nc.vector.tensor_copy(recip_b[:], rb_ps[:])
        attn_nT = consts.tile([128, HPC * B], f16)
        nc.vector.tensor_mul(attn_nT[:], aoT_sb[:], recip_b[:])

        # ---- output projection (transposed): outT[oc*128+p, b] ----
        # lhsT = ow chunk [128(hd%128), 128(outcol)], rhs = attn_nT slice [128(hd), B]
        for jg in range(5):  # groups of 8 outcol chunks (1024 cols)
            ops = [psum.tile([128, B], f32, tag="ps", name=f"ps_o{jg}_{i}") for i in range(8)]
            for hh in range(HPC):
                for oc in range(8):
                    j0 = jg * 1024 + oc * 128
                    nc.tensor.matmul(
                        ops[oc][:],
                        lhsT=ow_sb[:, hh * E + j0: hh * E + j0 + 128],
                        rhs=attn_nT[:, hh * B:(hh + 1) * B],
                        start=(hh == 0),
                        stop=(hh == HPC - 1),
                    )
            for oc in range(8):
                g_oc = jg * 8 + oc
                nc.scalar.copy(outT_sb[:, g_oc * B:(g_oc + 1) * B], ops[oc][:])

        nc.sync.dma_start(out=outT[:], in_=outT_sb[:])

    nc.compile()  # Bacc finalize: splits multi-waits (matmul 1-wait limit)
    return nc


def _prepare_core_inputs(core, hidden16, qkv16, o16, k16, v16, bt, sl, pos, nch):
    """Per-core staged arrays. hidden16/qkv16/o16/k16/v16 are fp16 full
    tensors (cast once by kernel()); this slices + packs layouts."""
    hs = slice(core * HPC, (core + 1) * HPC)
    es = slice(core * EPC, (core + 1) * EPC)

    # partition-major: qkvw[w, p, kc, c] = W[w, kc*128 + p, c] so each
    # DMA group reads GK*640*2 = 12.8KB contiguous per partition
    qkvw = np.ascontiguousarray(
        qkv16[:, :, es].reshape(3, 40, 128, EPC).transpose(0, 2, 1, 3)
    )

    sumnb = sum(nch)
    sumsd = sumnb * 128
    offc = [0]
    for n in nch:
        offc.append(offc[-1] + n)

    kg = k16[:, hs]  # [NB, HPC, BS, D]
    vg = v16[:, hs]
    kt = np.zeros((HPC, D, sumsd), np.float16)
    vt = np.zeros((HPC, 128, sumnb, D), np.float16)
    for b in range(B):
        sd = nch[b] * 128
        blocks = bt[b][: (sd + BS - 1) // BS]  # blocks covering the used chunks
        kk = kg[blocks].transpose(1, 0, 2, 3).reshape(HPC, sd, D)
        kt[:, :, offc[b] * 128: offc[b] * 128 + sd] = kk.transpose(0, 2, 1)
        vv = vg[blocks].transpose(1, 0, 2, 3).reshape(HPC, sd, D)
        vt[:, :, offc[b]: offc[b] + nch[b], :] = vv.reshape(
            HPC, nch[b], 128, D
        ).transpose(0, 2, 1, 3)

    slopes = _alibi_slopes(H)[core * HPC:(core + 1) * HPC]
    t_in = np.arange(128)[:, None]
    tg = (np.arange(16)[None, :] * 128 + t_in).astype(np.float32)  # [128, 16]
    biasa = np.empty((128, B, HPC, 16), np.float32)
    for b in range(B):
        for h in range(HPC):
            val = slopes[h] * (tg - np.float32(pos[b]))
            val[tg >= sl[b]] = NEG
            biasa[:, b, h, :] = val

    hTf = np.ascontiguousarray(
        hidden16.T.reshape(40, 128, B).transpose(1, 0, 2).reshape(128, 40 * B)
    )

    # ow pre-transposed: owr[p, h*E + j] = o_proj_weight[core*EPC + h*128 + p, j]
    owr = np.ascontiguousarray(
        o16[es].reshape(HPC, 128, E).transpose(1, 0, 2).reshape(128, HPC * E)
    )

    return dict(
        hT=hTf,
        qkvw=qkvw,
        ow=owr,
        kt=kt,
        vt=vt,
        bias=np.ascontiguousarray(biasa.reshape(128, B * HPC * 16)),
    )


def kernel(**inputs):
    global LAST_RESULTS
    hidden = np.asarray(inputs["hidden_states"], np.float32)
    qkv_w = np.asarray(inputs["qkv_weight"], np.float32)
    o_w = np.asarray(inputs["o_proj_weight"], np.float32)
    k_cache = np.asarray(inputs["k_cache"], np.float32)
    v_cache = np.asarray(inputs["v_cache"], np.float32)
    bt = np.asarray(inputs["block_tables"]).astype(np.int64)
    sl = np.asarray(inputs["sequence_lengths"]).astype(np.int64)

    pos = tuple(int(x) - 1 for x in sl)
    nch = tuple(int(math.ceil(int(x) / 128)) for x in sl)

    # cast once to fp16 (q pre-scaled by 1/sqrt(D) before the cast)
    hidden16 = hidden.astype(np.float16)
    qkv16 = qkv_w.copy()
    qkv16[0] *= np.float32(D ** -0.5)
    qkv16 = qkv16.astype(np.float16)
    o16 = o_w.astype(np.float16)
    k16 = k_cache.astype(np.float16)
    v16 = v_cache.astype(np.float16)

    in_maps = [
        _prepare_core_inputs(c, hidden16, qkv16, o16, k16, v16, bt, sl, pos, nch)
        for c in range(NCORES)
    ]

    key = (pos, nch)
    if key not in _PROGRAM_CACHE:
        _PROGRAM_CACHE[key] = _build_program(pos, nch)
    nc = _PROGRAM_CACHE[key]

    from concourse.bass_utils import run_bass_kernel_spmd

    res = run_bass_kernel_spmd(
        nc,
        in_maps,
        core_ids=list(range(NCORES)),
        trace=bool(os.environ.get("BASS_TRACE")),
    )
    LAST_RESULTS = res

    out = np.zeros((B, E), np.float64)
    for c in range(NCORES):
        r = np.asarray(res.results[c]["outT"])
        out += r.reshape(128, 40, B).transpose(2, 1, 0).reshape(B, E).astype(np.float64)
    return out.astype(np.float32)
